# revision 1
# baseline (speedup 1.0000x reference)
"""Causal self-attention (B=1, L=4096, C=1024, H=16, D=64) on 8 TRN2 NeuronCores.

Sharding: head tensor-parallelism - each core owns 2 of the 16 heads and
computes a full [C, L] partial of out.T = Wo_local.T @ attn_local.T; the
host sums the 8 partials.

All value-path tensors stay >= bf16: fp8 was measured (CoreSim) to cost
~3% rel err PER fp8 tensor (random-sign contractions don't average
quantization noise down), blowing the 2e-2 budget. fp16 is used where it
is free (same PE rate as bf16): qt/kt (S inputs), the reciprocal
broadcast (fp32 would run the PE at 4 cycles/row).

Per-core pipeline:
  QT/KT [128, L] fp16 = W.T @ xT  (psum fp32 -> fp16 copies)
  VT [128, L] bf16 -> PE-transpose per 128-block -> vaug [128, NKB, 130]
     bf16 ([h*65 : h*65+65] = [V_h | ones]): the PV matmul's 65th row
     accumulates the softmax denominator for free.
  per 512-wide q-block j, per k-block PAIR p (2 x 128 keys), per head:
    2 S^T matmuls into st [128, 2(kslab), 512] psum; ONE exp activation
    over both slabs (scale=1/8, bias=-2: no max-subtraction needed,
    |scores/8| < ~6 here, and the bias cancels in normalization while
    halving activation-instruction count);
    diagonal slabs: gpsimd memset kills the unwritten region, gpsimd
    affine_select zeroes the in-block upper triangle of pt - no additive
    mask pass on the DVE, exp of masked cols never computed as -30000;
    PV per slab: o_ps[h][65, 512] += vaug_i.T @ pt_slab (bf16).
  normalize: reciprocal_approx_fast (custom DVE op, ~5x faster than the
    iterative reciprocal) on the denominator row read straight from
    psum; fp16 outer-product broadcast on the PE; multiply into
    att2 [128, 512] bf16.
  Wo: 8 bf16 matmuls -> psum -> bf16 ot -> one batched DMA per j.

Software pipelining: PV(p-1) is emitted after S(p) so the in-order PE
queue never blocks on the scalar engine's exp; the V-transposes of
block b-1 are emitted after the projections of block b for the same
reason. x streams in per 512-column block (all C-chunks per DMA) so the
first attention block starts after ~1 MB, not after the full 8 MB.
"""
import math
import sys
from contextlib import ExitStack

import numpy as np

sys.path.insert(0, "/opt/trn_rl_repo")

import ml_dtypes  # noqa: E402

import concourse.bass as bass  # noqa: E402,F401
import concourse.mybir as mybir  # noqa: E402
import concourse.tile as tile  # noqa: E402
from concourse import bacc  # noqa: E402

FP32 = mybir.dt.float32
FP16 = mybir.dt.float16
BF16 = mybir.dt.bfloat16

L, C, H, D = 4096, 1024, 16, 64
N_CORES = 8
EXP_BIAS = -2.0


def _build_nc():
    DH2, QB, KB = 128, 512, 128
    NQ = L // QB          # 8 q-blocks
    NCC = C // 128        # 8 contraction chunks
    SUB = QB // KB        # 4 k-blocks per q-block width
    NKB = L // KB         # 32 k-blocks
    scale = 1.0 / math.sqrt(D)
    Exp = mybir.ActivationFunctionType.Exp

    nc = bacc.Bacc("TRN2", target_bir_lowering=False, debug=False,
                   num_devices=N_CORES)
    xT = nc.declare_dram_parameter("xT", [C, L], BF16, isOutput=False)
    # [128, NCC*DH2] chunk-major (host pre-reshaped)
    wq = nc.declare_dram_parameter("wq", [128, C], BF16, isOutput=False)
    wk = nc.declare_dram_parameter("wk", [128, C], BF16, isOutput=False)
    wv = nc.declare_dram_parameter("wv", [128, C], BF16, isOutput=False)
    wo = nc.declare_dram_parameter("wo", [DH2, C], BF16, isOutput=False)
    outT = nc.declare_dram_parameter("outT", [C, L], BF16, isOutput=True)

    xT_v = xT.rearrange("(n p) l -> p n l", n=NCC)
    outT_v = outT.rearrange("(n p) l -> p n l", n=NCC)

    with tile.TileContext(nc) as tc, ExitStack() as ctx:
        big = ctx.enter_context(tc.tile_pool(name="big", bufs=1))
        work = ctx.enter_context(tc.tile_pool(name="work", bufs=4))
        psA = ctx.enter_context(tc.tile_pool(name="psA", bufs=2, space="PSUM"))
        psS = ctx.enter_context(tc.tile_pool(name="psS", bufs=2, space="PSUM"))
        psO = ctx.enter_context(tc.tile_pool(name="psO", bufs=1, space="PSUM"))

        ident = big.tile([128, 128], BF16, tag="ident")
        nc.gpsimd.memset(ident[:], 0.0)
        nc.gpsimd.affine_select(out=ident[:], in_=ident[:],
                                compare_op=mybir.AluOpType.not_equal,
                                fill=1.0, base=0,
                                pattern=[[-1, 128]], channel_multiplier=1)
        ones64 = big.tile([1, D], FP16, tag="ones64")
        nc.gpsimd.memset(ones64[:], 1.0)
        ebias = big.tile([128, 1], FP32, tag="ebias")
        nc.gpsimd.memset(ebias[:], EXP_BIAS)

        xt_sb = big.tile([128, NCC, L], BF16, tag="xt")
        for b in range(NQ):
            nc.sync.dma_start(xt_sb[:, :, b * QB:(b + 1) * QB],
                              xT_v[:, :, b * QB:(b + 1) * QB])
        wq_sb = big.tile([128, NCC, DH2], BF16, tag="wq")
        wk_sb = big.tile([128, NCC, DH2], BF16, tag="wk")
        wv_sb = big.tile([128, NCC, DH2], BF16, tag="wv")
        for w_sb, w_dram in ((wq_sb, wq), (wk_sb, wk), (wv_sb, wv)):
            nc.sync.dma_start(
                w_sb[:], w_dram.rearrange("p (n d) -> p n d", n=NCC))
        wo_sb = big.tile([128, C], BF16, tag="wo")
        nc.sync.dma_start(wo_sb[:], wo[:])

        qt2 = big.tile([128, L], FP16, tag="qt2")
        kt2 = big.tile([128, L], FP16, tag="kt2")
        vt2 = big.tile([128, L], BF16, tag="vt2")
        vaug = big.tile([128, NKB, 130], BF16, tag="vaug")
        nc.gpsimd.memset(vaug[:, :, 64:65], 1.0)
        nc.gpsimd.memset(vaug[:, :, 129:130], 1.0)

        def emit_vtrans(b):
            # PE-transpose the 4 [128,128] bf16 V blocks of b into vaug
            for i in range(b * SUB, (b + 1) * SUB):
                trp = psA.tile([128, KB], BF16, tag="ps")
                nc.tensor.transpose(trp[:], vt2[:, i * KB:(i + 1) * KB],
                                    ident[:])
                nc.vector.tensor_copy(vaug[:, i, 0:64], trp[:, 0:64])
                nc.vector.tensor_copy(vaug[:, i, 65:129], trp[:, 64:128])

        for b in range(NQ):
            cols = slice(b * QB, (b + 1) * QB)
            for dst, w_sb in ((qt2, wq_sb), (kt2, wk_sb), (vt2, wv_sb)):
                pp = psA.tile([128, QB], FP32, tag="ps")
                for c in range(NCC):
                    nc.tensor.matmul(pp[:], w_sb[:, c, :],
                                     xt_sb[:, c, cols],
                                     start=(c == 0), stop=(c == NCC - 1))
                nc.vector.tensor_copy(dst[:, cols], pp[:])
            if b > 0:
                emit_vtrans(b - 1)
        emit_vtrans(NQ - 1)

        for j in range(NQ):
            att2 = work.tile([128, QB], BF16, tag="att2", bufs=2)
            o_ps = [psO.tile([65, QB], FP32, tag=f"o{h}", name=f"o_ps{h}")
                    for h in range(2)]
            nk = (j + 1) * SUB
            npair = nk // 2
            pending = None  # (pts, pair_idx) awaiting PV emission

            def emit_pv(pts, p):
                for h in range(2):
                    for s in range(2):
                        i = 2 * p + s
                        nc.tensor.matmul(
                            o_ps[h][:, :],
                            vaug[:, i, 65 * h:65 * h + 65],
                            pts[h][:, s, :],
                            start=(i == 0), stop=(i == nk - 1))

            for p in range(npair):
                c0s = [max(0, (2 * p + s - j * SUB)) * KB for s in range(2)]
                pts = []
                for h in range(2):
                    r0, r1 = h * D, (h + 1) * D
                    st = psS.tile([128, 2, QB], FP32, tag="st", name="st")
                    for s in range(2):
                        i = 2 * p + s
                        nc.tensor.matmul(
                            st[:, s, c0s[s]:QB],
                            kt2[r0:r1, i * KB:(i + 1) * KB],
                            qt2[r0:r1, j * QB + c0s[s]:(j + 1) * QB],
                            start=True, stop=True)
                    pt = work.tile([128, 2, QB], BF16, tag=f"pt{h}", bufs=3,
                                   name=f"pt{h}")
                    for s in range(2):
                        nc.scalar.activation(pt[:, s, c0s[s]:QB],
                                             st[:, s, c0s[s]:QB], Exp,
                                             bias=ebias[:], scale=scale)
                    for s in range(2):
                        if 2 * p + s >= j * SUB:
                            c0 = c0s[s]
                            if c0 > 0:
                                nc.gpsimd.memset(pt[:, s, 0:c0], 0.0)
                            nc.gpsimd.affine_select(
                                out=pt[:, s, c0:c0 + KB],
                                in_=pt[:, s, c0:c0 + KB],
                                compare_op=mybir.AluOpType.is_ge, fill=0.0,
                                base=0, pattern=[[1, KB]],
                                channel_multiplier=-1)
                    pts.append(pt)
                if pending is not None:
                    emit_pv(*pending)
                pending = (pts, p)
            if pending is not None:
                emit_pv(*pending)

            for h in range(2):
                r0, r1 = h * D, (h + 1) * D
                dn_sb = work.tile([1, QB], FP32, tag="dn")
                nc.vector.tensor_copy(dn_sb[:], o_ps[h][64:65, :])
                recip = work.tile([1, QB], FP32, tag="recip")
                nc.vector.reciprocal_approx_fast(recip[:], dn_sb[:])
                recip16 = work.tile([1, QB], FP16, tag="recip16")
                nc.vector.tensor_copy(recip16[:], recip[:])
                bc_ps = psA.tile([64, QB], FP32, tag="ps")
                nc.tensor.matmul(bc_ps[:], ones64[:], recip16[:],
                                 start=True, stop=True)
                bc_sb = work.tile([64, QB], FP32, tag="bc")
                nc.vector.tensor_copy(bc_sb[:], bc_ps[:])
                nc.vector.tensor_mul(att2[r0:r1, :], o_ps[h][0:64, :],
                                     bc_sb[:])

            ot = work.tile([128, NCC, QB], BF16, tag="ot", bufs=2)
            for cc in range(NCC):
                op = psA.tile([128, QB], FP32, tag="ps")
                nc.tensor.matmul(op[:], wo_sb[:, cc * 128:(cc + 1) * 128],
                                 att2[:], start=True, stop=True)
                nc.vector.tensor_copy(ot[:, cc, :], op[:])
            nc.sync.dma_start(outT_v[:, :, j * QB:(j + 1) * QB], ot[:])
    nc.compile()
    return nc


_NC_CACHE = None


def _get_nc():
    global _NC_CACHE
    if _NC_CACHE is None:
        _NC_CACHE = _build_nc()
    return _NC_CACHE


def _chunk_major(w):
    """[1024, 128] -> [128, 8*128]: element [p, n*128+d] = w[n*128+p, d]."""
    return np.ascontiguousarray(
        w.reshape(8, 128, 128).transpose(1, 0, 2).reshape(128, 1024))


def make_in_maps(x, Wq, Wk, Wv, Wo):
    bf16 = ml_dtypes.bfloat16
    x = np.asarray(x, np.float32).reshape(L, C)
    xT = np.ascontiguousarray(x.T).astype(bf16)
    Wq, Wk, Wv, Wo = (np.asarray(w, np.float32) for w in (Wq, Wk, Wv, Wo))
    in_maps = []
    for c in range(N_CORES):
        cols = slice(128 * c, 128 * (c + 1))
        in_maps.append({
            "xT": xT,
            "wq": _chunk_major(Wq[:, cols]).astype(bf16),
            "wk": _chunk_major(Wk[:, cols]).astype(bf16),
            "wv": _chunk_major(Wv[:, cols]).astype(bf16),
            "wo": np.ascontiguousarray(Wo[cols, :]).astype(bf16),
        })
    return in_maps


def combine_results(results):
    acc = np.zeros((C, L), np.float32)
    for r in results:
        acc += np.asarray(r["outT"], np.float32)
    return np.ascontiguousarray(acc.T)[None].astype(np.float32)


def kernel(x, Wq, Wk, Wv, Wo):
    from concourse.bass_utils import run_bass_kernel_spmd
    nc = _get_nc()
    in_maps = make_in_maps(x, Wq, Wk, Wv, Wo)
    res = run_bass_kernel_spmd(nc, in_maps, core_ids=list(range(N_CORES)))
    return combine_results(res.results)

